# revision 13
# baseline (speedup 1.0000x reference)
"""Causal self-attention (GQA + RoPE) Trainium2 Bass kernel, 8-way sharded.

Sharding: core c -> batch b = c // 2, head-half hh = c % 2.
Each core computes qkv projection, attention and output projection for its
batch and its 16 query heads / 4 kv heads; the output projection is a
row-shard of Wproj, so the two cores of a batch produce partial sums that
the host adds.

Design (all-SBUF, bf16; TimelineSim ~636us/core, PE ~97% busy at ~97% of
the 1-col/cycle matmul roofline):
  - q/k projection runs W-stationary (moving operand = xT), so the PE emits
    q^T/k^T [hd, T] directly -- no PE transposes, no DRAM round trip. RoPE
    (rotate-half form via host-side column de-interleave) is applied on the
    PSUM->SBUF path by ACT (half swap) + DVE (ps*cos) + Pool (sw*sin) +
    DVE (add, bf16 out).
  - v runs x-stationary into natural [T, 4*HD] layout; its 8 psum chains
    ride inside the group-0 slots (3 concurrently during the DMA-paced
    startup window, in per-x-chunk lockstep with the k0 chain).
  - attention uses transposed scores (scoresT = kT_slice^T @ qT) so no
    transposes are needed anywhere. Softmax: exp on ACT (PSUM->SBUF, bf16),
    si-pair exp tiles tree-summed on DVE, then ONE ones-matrix matmul per
    256-col chunk yields the denominator already broadcast across
    partitions (replaces per-si reduction matmuls + broadcast matmuls).
    The odd-diagonal 128-block is trimmed to half-width matmuls/exp.
  - attention for kv-group g is emitted interleaved (fine-grained, between
    qkv cc-steps -- engine queues are FIFO, so emission order IS schedule)
    into the qkv slots of group g+1; group 3 additionally schedules into
    its own late slots, and its tail overlaps the first three projection
    accumulations (ycc 0-11, which don't depend on the last heads).
  - projection reads SBUF-resident bf16 y^T tiles; everything bf16 except
    PSUM accumulation (fp32) and the final output.
"""

import os

os.environ.setdefault("JAX_PLATFORMS", "axon")

import numpy as np
import ml_dtypes

BF16 = ml_dtypes.bfloat16

B, T, C = 4, 1024, 4096
H, KV, HD = 32, 8, 128
REP = H // KV  # 4

NQ = 16      # q heads per core
NKV = 4      # kv heads per core
QK_HEADS = NQ + NKV   # 20 projected+rope'd heads per core
NCC = C // 128        # 32 contraction tiles
NDR = C // 256        # 16 DoubleRow passes (256-contraction each)
NTT = T // 128        # 8 token tiles
SCALE = float(1.0 / np.sqrt(np.float32(HD)).astype(np.float32))
# fp8 scaling for the q/k projection: x and Wqk are multiplied by QK_FS
# before e4m3 quantization; the exp() scale divides the product back out.
QK_FS = 64.0
# qkT tiles are stored fp8 as well; csg/ssg carry an extra 1/QK_FS2 so the
# rope'd q/k (std ~0.026 * QK_FS**2 / QK_FS2 ~ 13) fit e4m3 range.
QK_FS2 = 8.0
EXP_SCALE = float(SCALE * QK_FS2**2 / QK_FS**4)

_CACHE: dict = {}


def _build_nc():
    import concourse.mybir as mybir
    import concourse.tile as tile
    from concourse import bacc
    from concourse.bass import ts

    f32 = mybir.dt.float32
    bf16 = mybir.dt.bfloat16
    f8 = mybir.dt.float8e4
    DRow = mybir.MatmulPerfMode.DoubleRow
    Exp = mybir.ActivationFunctionType.Exp

    nc = bacc.Bacc(None, target_bir_lowering=False, debug=False)

    xT_d = nc.dram_tensor("xT", [C, T], bf16, kind="ExternalInput")
    # fp8 x for the q/k projection, pre-paired for DoubleRow:
    # x8[p, j, i, t] = fp8(QK_FS * x[t, 256*j + 128*i + p])
    x8_d = nc.dram_tensor("x8", [128, NDR, 2, T], f8, kind="ExternalInput")
    # [h, p(c within pair), pass, i, col]
    wqk_d = nc.dram_tensor("wqk", [QK_HEADS, 128, NDR, 2, 128], f8,
                           kind="ExternalInput")
    # [p, cc, vcol]
    wv_d = nc.dram_tensor("wv", [128, NCC, NKV * HD], bf16, kind="ExternalInput")
    # [ccol, p(y within tile), ycc, f]
    wp_d = nc.dram_tensor("wp", [8, 128, 16, 512], bf16, kind="ExternalInput")
    csg_d = nc.dram_tensor("csg", [128, T], f8, kind="ExternalInput")
    ssg_d = nc.dram_tensor("ssg", [128, T], f8, kind="ExternalInput")
    mask_d = nc.dram_tensor("maskd", [128, 2, 256], bf16, kind="ExternalInput")
    out_d = nc.dram_tensor("out", [T, C], f32, kind="ExternalOutput")

    with (
        tile.TileContext(nc) as tc,
        tc.tile_pool(name="const", bufs=1) as const_p,
        tc.tile_pool(name="qkT", bufs=10) as qkT_p,
        tc.tile_pool(name="yts", bufs=NQ) as yts_p,
        tc.tile_pool(name="vsb", bufs=1) as vsb_p,
        tc.tile_pool(name="sw", bufs=1) as sw_p,
        tc.tile_pool(name="m2", bufs=1) as m2_p,
        tc.tile_pool(name="et", bufs=10) as et_p,
        tc.tile_pool(name="se", bufs=5) as se_p,
        tc.tile_pool(name="t2", bufs=2) as t2_p,
        tc.tile_pool(name="rb", bufs=1) as rb_p,
        tc.tile_pool(name="psQK", bufs=2, space="PSUM") as psQK,
        tc.tile_pool(name="psS", bufs=2, space="PSUM") as psS,
        tc.tile_pool(name="psY", bufs=1, space="PSUM") as psY,
    ):
        ones128 = const_p.tile([128, 128], bf16)
        nc.vector.memset(ones128[:], 1.0)
        csg = const_p.tile([128, T], f8)
        ssg = const_p.tile([128, T], f8)
        mask_sb = const_p.tile([128, 2, 256], bf16)

        v_sb = vsb_p.tile([128, NTT, NKV * HD], bf16)
        yts = [yts_p.tile([128, T], bf16, tag="yt", name=f"yt{i}")
               for i in range(NQ)]
        qkT: dict = {}

        # ---- attention emission helpers (interleaved into qkv slots) ----
        et_tiles: dict = {}
        se_tiles: dict = {}

        def scores_steps(g, r):
            """Emission-step closures: scoresT + exp + mask + pair-tree sums
            for q head (g, r). One step per si pair (10 total)."""
            hq = 4 * g + r
            ew = nc.vector

            def mk(c, pair):
                def step():
                    qt = qkT[hq]
                    kt = qkT[NQ + g]
                    np_ = c + 1
                    pss = psS.tile([128, 2, 256], f32, tag="psS")
                    et = et_p.tile([128, 2, 256], bf16, tag="et")
                    if pair == c:
                        # diagonal pair: si=2c covers all 256 cols, si=2c+1
                        # only the upper 128 (rest fully masked -> zeros)
                        nc.tensor.matmul(
                            pss[:, 0, :], kt[:, ts(2 * pair, 128)],
                            qt[:, ts(c, 256)], start=True, stop=True,
                        )
                        nc.tensor.matmul(
                            pss[:, 1, 128:256], kt[:, ts(2 * pair + 1, 128)],
                            qt[:, 256 * c + 128 : 256 * c + 256],
                            start=True, stop=True,
                        )
                        nc.vector.memset(et[:, 1, 0:128], 0.0)
                        nc.scalar.activation(
                            out=et[:, 0, :], in_=pss[:, 0, :], func=Exp,
                            scale=EXP_SCALE,
                        )
                        nc.scalar.activation(
                            out=et[:, 1, 128:256], in_=pss[:, 1, 128:256],
                            func=Exp, scale=EXP_SCALE,
                        )
                    else:
                        for i in range(2):
                            si = 2 * pair + i
                            nc.tensor.matmul(
                                pss[:, i, :],
                                kt[:, ts(si, 128)],
                                qt[:, ts(c, 256)],
                                start=True,
                                stop=True,
                            )
                        nc.scalar.activation(
                            out=et[:], in_=pss[:], func=Exp, scale=EXP_SCALE,
                        )
                    pairs = et_tiles.setdefault((hq, c), [])
                    pairs.append(et)
                    if pair == c:  # diagonal pair: mask, then finish the sum
                        ew.tensor_mul(et[:], et[:], mask_sb[:])
                        se = se_p.tile([128, 256], bf16, tag="se")
                        if np_ == 1:
                            ew.tensor_add(se[:], et[:, 0, :], et[:, 1, :])
                        else:
                            t2 = t2_p.tile([128, 2, 256], bf16, tag="t2")
                            ew.tensor_add(t2[:], pairs[0][:], pairs[1][:])
                            for k in range(2, np_):
                                ew.tensor_add(t2[:], t2[:], pairs[k][:])
                            ew.tensor_add(se[:], t2[:, 0, :], t2[:, 1, :])
                        se_tiles[(hq, c)] = se
                return step

            return [mk(c, pair) for c in range(4) for pair in range(c + 1)]

        def avdn_steps(g, r):
            """Emission-step closures: AV + broadcast denominator + normalize
            for q head (g, r). One step per chunk plus one per chunk-pair."""
            hq = 4 * g + r
            state: dict = {}

            def mk_av(c):
                def step():
                    c2, i = divmod(c, 2)
                    if i == 0:
                        state["py"] = psY.tile([128, 2, 256], f32, tag="psY", name="py")
                        state["dn"] = psS.tile([128, 2, 256], f32, tag="psS", name="dn")
                    py, dn = state["py"], state["dn"]
                    ns = 2 * (c + 1)
                    pairs = et_tiles.pop((hq, c))
                    for si in range(ns):
                        if si == ns - 1:
                            # odd-diagonal block: lower 128 tq cols are zero
                            nc.tensor.matmul(
                                py[:, i, 128:256],
                                v_sb[:, si, ts(g, 128)],
                                pairs[si // 2][:, si % 2, 128:256],
                                start=False,
                                stop=True,
                            )
                            continue
                        nc.tensor.matmul(
                            py[:, i, :],
                            v_sb[:, si, ts(g, 128)],
                            pairs[si // 2][:, si % 2, :],
                            start=(si == 0),
                            stop=False,
                        )
                    se = se_tiles.pop((hq, c))
                    nc.tensor.matmul(
                        dn[:, i, :], ones128[:], se[:], start=True, stop=True
                    )
                return step

            def mk_norm(c2):
                def step():
                    py, dn = state["py"], state["dn"]
                    rb = rb_p.tile([128, 2, 256], f32, tag="rb")
                    nc.vector.reciprocal(out=rb[:], in_=dn[:])
                    nc.vector.tensor_mul(
                        yts[hq][:, ts(c2, 512)].rearrange(
                            "p (a b) -> p a b", a=2),
                        py[:], rb[:],
                    )
                return step

            out = []
            for c2 in range(2):
                out.append(mk_av(2 * c2))
                out.append(mk_av(2 * c2 + 1))
                out.append(mk_norm(c2))
            return out

        def interleave(primary, inserts):
            """Emit primary closures with inserts spread evenly between."""
            n, m = len(primary), len(inserts)
            ii = 0
            for k, p in enumerate(primary):
                p()
                want = (k + 1) * m // n
                while ii < want:
                    inserts[ii]()
                    ii += 1
            while ii < m:
                inserts[ii]()
                ii += 1

        # ================= PHASE A+B: qkv + interleaved attention =========
        with (
            tc.tile_pool(name="x", bufs=1) as x_p,
            tc.tile_pool(name="wv", bufs=4) as wv_p,
            tc.tile_pool(name="wqk", bufs=3) as wqk_p,
        ):
            # head order per group: [k_g, q_{4g}, .., q_{4g+3}]
            def head_list(g):
                return [NQ + g, 4 * g, 4 * g + 1, 4 * g + 2, 4 * g + 3]

            wqk_t: dict = {}

            def fetch_w(h, half):
                wqk_t[(h, half)] = wqk_p.tile(
                    [128, 8, 2, 128], f8, tag="wqk", name=f"wqk{h}_{half}")
                nc.sync.dma_start(
                    out=wqk_t[(h, half)][:], in_=wqk_d[h, :, ts(half, 8), :, :])

            # DMA priority order, staggered so the k0/v-tt0 startup chains
            # are fed as early as possible: k0 W, then alternating x chunks
            # and v weights, then the small constants.
            fetch_w(head_list(0)[0], 0)
            xT_sb = x_p.tile([128, NCC, T], bf16)
            x8_sb = x_p.tile([128, NDR, 2, T], f8)
            xr = xT_d.rearrange("(cc p) t -> p cc t", p=128)

            def fetch_x8(ch):
                nc.sync.dma_start(
                    out=x8_sb[:, ts(ch, 2)], in_=x8_d[:, ts(ch, 2)]
                )

            def fetch_x(ch):
                nc.sync.dma_start(
                    out=xT_sb[:, ts(ch, 4), :], in_=xr[:, ts(ch, 4), :]
                )

            wv_t = {}

            def fetch_wv(ch):
                wv_t[ch] = wv_p.tile([128, 8, NKV * HD], bf16, tag="wv",
                                     name=f"wv{ch}")
                nc.sync.dma_start(out=wv_t[ch][:], in_=wv_d[:, ts(ch, 8), :])

            fetch_x8(0)
            fetch_x(0)
            fetch_wv(0)
            fetch_x8(1)
            fetch_x(1)
            fetch_w(head_list(0)[0], 1)
            fetch_x8(2)
            fetch_x(2)
            fetch_wv(1)
            fetch_x8(3)
            fetch_x(3)
            fetch_x8(4)
            fetch_x(4)
            fetch_wv(2)
            fetch_x8(5)
            fetch_x(5)
            fetch_x8(6)
            fetch_x(6)
            fetch_x8(7)
            fetch_x(7)
            fetch_wv(3)
            nc.sync.dma_start(out=csg[:], in_=csg_d[:])
            nc.sync.dma_start(out=ssg[:], in_=ssg_d[:])
            nc.sync.dma_start(out=mask_sb[:], in_=mask_d[:])

            # v psum chains cycle through the two psS slots (idle during
            # group 0) plus the dedicated psV bank -> 3 concurrent chains
            v_pool_cycle = ["psS", "psS", "psV"]

            def v_steps(tt):
                """One v token-tile: 32-step accumulation + PSUM->SBUF copy."""
                vtag = v_pool_cycle[tt % 3]
                psv = (psS if vtag == "psS" else psQK).tile(
                    [128, 512], f32, tag=vtag, bufs=(1 if vtag == "psV" else None),
                    name=f"psv{tt}")

                def mk_vcc(cc):
                    def step():
                        nc.tensor.matmul(
                            psv[:],
                            xT_sb[:, cc, ts(tt, 128)],
                            wv_t[cc // 8][:, cc % 8, :],
                            start=(cc == 0),
                            stop=(cc == NCC - 1),
                        )
                        if cc == NCC - 1:
                            nc.scalar.copy(out=v_sb[:, tt, :], in_=psv[:])
                    return step

                return [mk_vcc(cc) for cc in range(NCC)]

            def emit_slot(g, qk_steps, ins):
                if g == 0:
                    # per-chunk lockstep: all chains consume x chunk m
                    # before any chain touches chunk m+1 (DMA pacing).
                    # qk (DoubleRow) advances 2 passes per chunk; v chains
                    # advance 4 cc steps per chunk.
                    for ch in range(8):
                        for s in qk_steps[2 * ch : 2 * ch + 2]:
                            s()
                        for chain in ins:
                            for s in chain[4 * ch : 4 * ch + 4]:
                                s()
                else:
                    interleave(qk_steps, ins)

            # ---- q/k heads in groups; attention + v interleaved into slots
            def slot_inserts(g, j):
                ins = []
                if g == 0:
                    # v tiles: tt0-tt2 ride with k0 (fills the DMA-paced
                    # startup window), the rest spread over the q slots.
                    # Returned as chains for per-chunk lockstep emission.
                    vmap = {0: [0, 1, 2], 1: [3, 4], 2: [5], 3: [6], 4: [7]}
                    return [v_steps(tt) for tt in vmap[j]]
                # attention of the previous group: head j hosted in slot j
                # (the j=4 slot stays single-loaded for g3's early heads)
                if g == 3 and j >= 3:
                    ins += avdn_steps(3, j - 3)
                if j <= 3:
                    ins += scores_steps(g - 1, j)
                    ins += avdn_steps(g - 1, j)
                if g == 3 and j >= 2:
                    ins += scores_steps(3, j - 2)
                return ins

            for g in range(4):
                heads = head_list(g)
                for j, h in enumerate(heads):
                    # prefetch: this head's second half + next head's first
                    if (h, 1) not in wqk_t:
                        fetch_w(h, 1)
                    nxt = heads[j + 1] if j + 1 < 5 else (
                        head_list(g + 1)[0] if g + 1 < 4 else None)
                    if nxt is not None:
                        fetch_w(nxt, 0)
                    ps = psQK.tile([128, T], f32, tag="psQK")
                    wt0 = wqk_t.pop((h, 0))
                    wt1 = wqk_t.pop((h, 1))

                    def mk_pass(jj, wt, base):
                        def step():
                            nc.tensor.matmul(
                                ps[:, 0:512], wt[:, jj - base],
                                x8_sb[:, jj, :, 0:512],
                                start=(jj == 0), stop=(jj == NDR - 1),
                                perf_mode=DRow,
                            )
                            nc.tensor.matmul(
                                ps[:, 512:1024], wt[:, jj - base],
                                x8_sb[:, jj, :, 512:1024],
                                start=(jj == 0), stop=(jj == NDR - 1),
                                perf_mode=DRow,
                            )
                        return step

                    qk_steps = [mk_pass(jj, wt0, 0) for jj in range(8)]
                    qk_steps += [mk_pass(jj, wt1, 8) for jj in range(8, NDR)]
                    emit_slot(g, qk_steps, slot_inserts(g, j))
                    # RoPE: out = ps*csg + swap_halves(ps)*ssg
                    sw = sw_p.tile([128, T], bf16, tag="sw")
                    nc.scalar.copy(out=sw[0:64, :], in_=ps[64:128, :])
                    nc.scalar.copy(out=sw[64:128, :], in_=ps[0:64, :])
                    m2 = m2_p.tile([128, T], bf16, tag="m2")
                    nc.gpsimd.tensor_mul(m2[:], sw[:], ssg[:])
                    qt = qkT_p.tile([128, T], f8, tag="qkT", name=f"qkT{h}")
                    nc.vector.tensor_mul(qt[:], ps[:], csg[:])
                    nc.vector.tensor_add(qt[:], qt[:], m2[:])
                    qkT[h] = qt

        # ================= PHASE C: last-group attention + proj ===========
        with (
            tc.tile_pool(name="wpp", bufs=4) as wp_p,
            tc.tile_pool(name="ostage", bufs=3) as ostage_p,
        ):
            wp_t: dict = {}

            def fetch_wp(ccol):
                lo = wp_p.tile([128, 8, 512], bf16, tag="wp", name=f"wpl{ccol}")
                hi = wp_p.tile([128, 8, 512], bf16, tag="wp", name=f"wph{ccol}")
                nc.sync.dma_start(out=lo[:], in_=wp_d[ccol, :, 0:8, :])
                nc.sync.dma_start(out=hi[:], in_=wp_d[ccol, :, 8:16, :])
                wp_t[ccol] = (lo, hi)

            fetch_wp(0)

            def po_steps(ccol, tt, po, ycc_range):
                lo, hi = wp_t[ccol]

                def mk(ycc):
                    def step():
                        wtile = lo if ycc < 8 else hi
                        nc.tensor.matmul(
                            po[:],
                            yts[ycc][:, ts(tt, 128)],
                            wtile[:, ycc % 8, :],
                            start=(ycc == 0),
                            stop=(ycc == 15),
                        )
                        if ycc != 15:
                            return
                        ot = ostage_p.tile([128, 512], f32, tag="os")
                        nc.scalar.copy(out=ot[:], in_=po[:])
                        nc.sync.dma_start(
                            out=out_d[ts(tt, 128), ts(ccol, 512)],
                            in_=ot[:],
                        )
                    return step

                return [mk(ycc) for ycc in ycc_range]

            # remaining attention tail (the rest rode inside the B slots);
            # the first three proj accumulations' ycc 0-11 matmuls (which do
            # not depend on the last attention heads) fill the exp stalls.
            po0 = psQK.tile([128, 512], f32, tag="psQK", name="po0")
            po1 = psQK.tile([128, 512], f32, tag="psQK", name="po1")
            po2 = psQK.tile([128, 512], f32, tag="psV", bufs=1, name="po2")
            tail = (avdn_steps(3, 2) + scores_steps(3, 3) + avdn_steps(3, 3))
            interleave(tail, po_steps(0, 0, po0, range(12))
                       + po_steps(0, 1, po1, range(12))
                       + po_steps(0, 2, po2, range(12)))

            first = {0: (po0, 12), 1: (po1, 12), 2: (po2, 12)}
            for ccol in range(8):
                if ccol + 1 < 8:
                    fetch_wp(ccol + 1)
                for tt in range(NTT):
                    po, ystart = None, 0
                    if ccol == 0 and tt in first:
                        po, ystart = first[tt]
                    else:
                        po = psQK.tile([128, 512], f32, tag="psQK", name="po")
                    for s in po_steps(ccol, tt, po, range(ystart, 16)):
                        s()
                wp_t.pop(ccol)

    nc.compile()
    return nc


F8 = ml_dtypes.float8_e4m3


def prep_inputs(x, Wqkv, Wproj, freqs_cos, freqs_sin):
    """Build the 8 per-core input maps (host-side shard + layout prep)."""
    x = np.asarray(x, np.float32)
    Wqkv = np.asarray(Wqkv, np.float32)
    Wproj = np.asarray(Wproj, np.float32)
    cos = np.asarray(freqs_cos, np.float32)
    sin = np.asarray(freqs_sin, np.float32)

    perm = np.concatenate([np.arange(0, HD, 2), np.arange(1, HD, 2)])
    csg = np.ascontiguousarray(
        (np.vstack([cos.T, cos.T]) / QK_FS2).astype(F8))   # [128, T]
    ssg = np.ascontiguousarray(
        (np.vstack([-sin.T, sin.T]) / QK_FS2).astype(F8))  # [128, T]
    # mask[p, i, f] = 1.0 if 128*i + p <= f else 0 (diagonal 256-chunk pair)
    mask = (
        (128 * np.arange(2)[None, :, None] + np.arange(128)[:, None, None])
        <= np.arange(256)[None, None, :]
    ).astype(BF16)
    mask = np.ascontiguousarray(mask)

    in_maps = []
    for c in range(8):
        b, hh = divmod(c, 2)
        qcols = (hh * NQ * HD + (np.arange(NQ) * HD)[:, None] + perm[None, :]).ravel()
        kcols = (
            H * HD + hh * NKV * HD + (np.arange(NKV) * HD)[:, None] + perm[None, :]
        ).ravel()
        vcols = (
            (H + KV) * HD
            + hh * NKV * HD
            + (np.arange(NKV) * HD)[:, None]
            + np.arange(HD)[None, :]
        ).ravel()
        Wqk = Wqkv[:, np.concatenate([qcols, kcols])]      # [4096, 2560]
        # fp8 DoubleRow layout [h, p, pass, i, col]:
        # contraction index c = 256*pass + 128*i + p
        wqk = np.ascontiguousarray(
            (QK_FS * Wqk).astype(F8)
            .reshape(NDR, 2, 128, QK_HEADS, 128).transpose(3, 2, 0, 1, 4))
        Wv = Wqkv[:, vcols]                                # [4096, 512]
        wv = np.ascontiguousarray(
            Wv.reshape(NCC, 128, NKV * HD).transpose(1, 0, 2).astype(BF16))
        Wp = Wproj[hh * NQ * HD : (hh + 1) * NQ * HD, :]   # [2048, 4096]
        wp = np.ascontiguousarray(
            Wp.reshape(16, 128, 8, 512).transpose(2, 1, 0, 3).astype(BF16))
        xT = np.ascontiguousarray(x[b].T.astype(BF16))     # [4096, 1024]
        x8 = np.ascontiguousarray(
            (QK_FS * x[b].T).astype(F8)
            .reshape(NDR, 2, 128, T).transpose(2, 0, 1, 3))  # [128, pass, i, T]
        in_maps.append(
            {"xT": xT, "x8": x8, "wqk": wqk, "wv": wv, "wp": wp,
             "csg": csg, "ssg": ssg, "maskd": mask}
        )
    return in_maps


def _get_nc():
    if "nc" not in _CACHE:
        _CACHE["nc"] = _build_nc()
    return _CACHE["nc"]


def kernel(x, Wqkv, Wproj, freqs_cos, freqs_sin, mask=None):
    from concourse.bass_utils import run_bass_kernel_spmd

    nc = _get_nc()
    in_maps = prep_inputs(x, Wqkv, Wproj, freqs_cos, freqs_sin)
    res = run_bass_kernel_spmd(nc, in_maps, core_ids=list(range(8)))
    outs = [res.results[c]["out"] for c in range(8)]
    y = np.stack([outs[2 * b] + outs[2 * b + 1] for b in range(B)], axis=0)
    return y.astype(np.float32)



# revision 15
# speedup vs baseline: 1.3119x; 1.3119x over previous
"""Causal self-attention (GQA + RoPE) Trainium2 Bass kernel, 8-way sharded.

Sharding: core c -> batch b = c // 2, head-half hh = c % 2.
Each core computes qkv projection, attention and output projection for its
batch and its 16 query heads / 4 kv heads; the output projection is a
row-shard of Wproj, so the two cores of a batch produce partial sums that
the host adds.

Design (all-SBUF):
  - q/k projection runs W-stationary in fp8-e4m3 DoubleRow (contraction 256
    per pass), emitting q^T/k^T [hd, T] directly. The fp8 quantization of
    x/Wqk/cos/sin/qkT only perturbs attention *scores* (softmax weights move
    by ~1e-4 relative for this data regime), so it is error-neutral; v and
    the projections stay bf16. Scale bookkeeping: x8 = fp8(64 x),
    w8 = fp8(64 W), csg/ssg carry 1/8, so qkT = 512 q_rope (fits e4m3
    range) and exp() divides by 512^2.
  - RoPE is applied per 512-col half on the PSUM->SBUF path: ACT (half
    swap) + Pool (sw*ssg) + DVE (ps*csg into qt, then += m2).
  - each q/k head accumulates its two 512-col halves in separate 1-bank
    PSUM tiles from a 3-ring, so RoPE of half 0 overlaps accumulation of
    half 1 and heads pipeline without bank stalls.
  - v runs x-stationary (bf16) into natural [T, 4*HD] layout; in group 0
    its 8 psum chains run up to 5-wide (3-ring + 2-ring banks) in per-x-
    chunk lockstep with k0, absorbing the DMA-paced startup window.
  - attention uses transposed scores (scoresT = kT_slice^T @ qT). Softmax:
    exp on ACT (PSUM->SBUF, bf16), si-pair exp tiles tree-summed on DVE,
    then ONE ones-matrix matmul per 256-col chunk yields the denominator
    broadcast across partitions. Score psums draw from a 3-ring; AV output
    and the denominator alternate in a separate 2-ring, so score matmuls
    never wait on the AV/normalize pipeline's banks.
  - attention for kv-group g is emitted interleaved into the qkv slots of
    group g+1; group 3's tail overlaps the first three projection
    accumulations (ycc 0-11).
  - projection reads SBUF-resident bf16 y^T tiles; PSUM accumulation fp32.
"""

import os

os.environ.setdefault("JAX_PLATFORMS", "axon")

import numpy as np
import ml_dtypes

BF16 = ml_dtypes.bfloat16
F8 = ml_dtypes.float8_e4m3

B, T, C = 4, 1024, 4096
H, KV, HD = 32, 8, 128
REP = H // KV  # 4

NQ = 16      # q heads per core
NKV = 4      # kv heads per core
QK_HEADS = NQ + NKV   # 20 projected+rope'd heads per core
NCC = C // 128        # 32 contraction tiles
NDR = C // 256        # 16 DoubleRow passes (256-contraction each)
NTT = T // 128        # 8 token tiles
SCALE = float(1.0 / np.sqrt(np.float32(HD)).astype(np.float32))
# fp8 scaling for the q/k projection: x and Wqk are multiplied by QK_FS
# before e4m3 quantization; qkT carries QK_FS**2/QK_FS2; exp() divides the
# product of two such factors back out.
QK_FS = 64.0
QK_FS2 = 8.0
EXP_SCALE = float(SCALE * QK_FS2**2 / QK_FS**4)

_CACHE: dict = {}


def _build_nc():
    import concourse.mybir as mybir
    import concourse.tile as tile
    from concourse import bacc
    from concourse.bass import ts

    f32 = mybir.dt.float32
    bf16 = mybir.dt.bfloat16
    f8 = mybir.dt.float8e4
    DRow = mybir.MatmulPerfMode.DoubleRow
    Exp = mybir.ActivationFunctionType.Exp

    nc = bacc.Bacc(None, target_bir_lowering=False, debug=False)

    xT_d = nc.dram_tensor("xT", [C, T], bf16, kind="ExternalInput")
    # fp8 x for the q/k projection, pre-paired for DoubleRow:
    # x8[p, j, i, t] = fp8(QK_FS * x[t, 256*j + 128*i + p])
    x8_d = nc.dram_tensor("x8", [128, NDR, 2, T], f8, kind="ExternalInput")
    # [h, p(c within pair), pass, i, col]
    wqk_d = nc.dram_tensor("wqk", [QK_HEADS, 128, NDR, 2, 128], f8,
                           kind="ExternalInput")
    # [p, cc, vcol]
    wv_d = nc.dram_tensor("wv", [128, NCC, NKV * HD], bf16, kind="ExternalInput")
    # [ccol, p(y within tile), ycc, f]
    wp_d = nc.dram_tensor("wp", [8, 128, 16, 512], bf16, kind="ExternalInput")
    csg_d = nc.dram_tensor("csg", [128, T], f8, kind="ExternalInput")
    ssg_d = nc.dram_tensor("ssg", [128, T], f8, kind="ExternalInput")
    mask_d = nc.dram_tensor("maskd", [128, 2, 256], bf16, kind="ExternalInput")
    out_d = nc.dram_tensor("out", [T, C], f32, kind="ExternalOutput")

    with (
        tile.TileContext(nc) as tc,
        tc.tile_pool(name="const", bufs=1) as const_p,
        tc.tile_pool(name="qkT", bufs=10) as qkT_p,
        tc.tile_pool(name="yts", bufs=NQ) as yts_p,
        tc.tile_pool(name="vsb", bufs=1) as vsb_p,
        tc.tile_pool(name="sw", bufs=2) as sw_p,
        tc.tile_pool(name="m2", bufs=2) as m2_p,
        tc.tile_pool(name="et", bufs=10) as et_p,
        tc.tile_pool(name="se", bufs=5) as se_p,
        tc.tile_pool(name="t2", bufs=2) as t2_p,
        tc.tile_pool(name="rb", bufs=1) as rb_p,
        # PSUM bank map (8 banks total):
        #   psQK: 3 x 1 bank -- q/k half-accumulators; proj po chains
        #   psSm: 3 x 1 bank -- score psums; v chains (g0); po0-2 tail
        #   psYD: 2 x 1 bank -- AV output / softmax denominator; v chains
        tc.tile_pool(name="psQK", bufs=3, space="PSUM") as psQK,
        tc.tile_pool(name="psSm", bufs=3, space="PSUM") as psSm,
        tc.tile_pool(name="psYD", bufs=2, space="PSUM") as psYD,
    ):
        ones128 = const_p.tile([128, 128], bf16)
        nc.vector.memset(ones128[:], 1.0)
        csg = const_p.tile([128, T], f8)
        ssg = const_p.tile([128, T], f8)
        mask_sb = const_p.tile([128, 2, 256], bf16)

        v_sb = vsb_p.tile([128, NTT, NKV * HD], bf16)
        yts = [yts_p.tile([128, T], bf16, tag="yt", name=f"yt{i}")
               for i in range(NQ)]
        qkT: dict = {}

        # ---- attention emission helpers (interleaved into qkv slots) ----
        et_tiles: dict = {}
        se_tiles: dict = {}

        def scores_steps(g, r):
            """Emission-step closures: scoresT + exp + mask + pair-tree sums
            for q head (g, r). One step per si pair (10 total)."""
            hq = 4 * g + r
            ew = nc.vector

            def mk(c, pair):
                def step():
                    qt = qkT[hq]
                    kt = qkT[NQ + g]
                    np_ = c + 1
                    pss = psSm.tile([128, 2, 256], f32, tag="psS")
                    et = et_p.tile([128, 2, 256], bf16, tag="et")
                    if pair == c:
                        # diagonal pair: si=2c covers all 256 cols, si=2c+1
                        # only the upper 128 (rest fully masked -> zeros)
                        nc.tensor.matmul(
                            pss[:, 0, :], kt[:, ts(2 * pair, 128)],
                            qt[:, ts(c, 256)], start=True, stop=True,
                        )
                        nc.tensor.matmul(
                            pss[:, 1, 128:256], kt[:, ts(2 * pair + 1, 128)],
                            qt[:, 256 * c + 128 : 256 * c + 256],
                            start=True, stop=True,
                        )
                        nc.vector.memset(et[:, 1, 0:128], 0.0)
                        nc.scalar.activation(
                            out=et[:, 0, :], in_=pss[:, 0, :], func=Exp,
                            scale=EXP_SCALE,
                        )
                        nc.scalar.activation(
                            out=et[:, 1, 128:256], in_=pss[:, 1, 128:256],
                            func=Exp, scale=EXP_SCALE,
                        )
                    else:
                        for i in range(2):
                            si = 2 * pair + i
                            nc.tensor.matmul(
                                pss[:, i, :],
                                kt[:, ts(si, 128)],
                                qt[:, ts(c, 256)],
                                start=True,
                                stop=True,
                            )
                        nc.scalar.activation(
                            out=et[:], in_=pss[:], func=Exp, scale=EXP_SCALE,
                        )
                    pairs = et_tiles.setdefault((hq, c), [])
                    pairs.append(et)
                    if pair == c:  # diagonal pair: mask, then finish the sum
                        ew.tensor_mul(et[:], et[:], mask_sb[:])
                        se = se_p.tile([128, 256], bf16, tag="se")
                        if np_ == 1:
                            ew.tensor_add(se[:], et[:, 0, :], et[:, 1, :])
                        else:
                            t2 = t2_p.tile([128, 2, 256], bf16, tag="t2")
                            ew.tensor_add(t2[:], pairs[0][:], pairs[1][:])
                            for k in range(2, np_):
                                ew.tensor_add(t2[:], t2[:], pairs[k][:])
                            ew.tensor_add(se[:], t2[:, 0, :], t2[:, 1, :])
                        se_tiles[(hq, c)] = se
                return step

            return [mk(c, pair) for c in range(4) for pair in range(c + 1)]

        def avdn_steps(g, r):
            """Emission-step closures: AV + broadcast denominator + normalize
            for q head (g, r). One step per chunk plus one per chunk-pair."""
            hq = 4 * g + r
            state: dict = {}

            def mk_av(c):
                def step():
                    c2, i = divmod(c, 2)
                    if i == 0:
                        state["py"] = psYD.tile([128, 2, 256], f32, tag="psY",
                                                name="py")
                        state["dn"] = psYD.tile([128, 2, 256], f32, tag="psY",
                                                name="dn")
                    py, dn = state["py"], state["dn"]
                    ns = 2 * (c + 1)
                    pairs = et_tiles.pop((hq, c))
                    for si in range(ns):
                        if si == ns - 1:
                            # odd-diagonal block: lower 128 tq cols are zero
                            nc.tensor.matmul(
                                py[:, i, 128:256],
                                v_sb[:, si, ts(g, 128)],
                                pairs[si // 2][:, si % 2, 128:256],
                                start=False,
                                stop=True,
                            )
                            continue
                        nc.tensor.matmul(
                            py[:, i, :],
                            v_sb[:, si, ts(g, 128)],
                            pairs[si // 2][:, si % 2, :],
                            start=(si == 0),
                            stop=False,
                        )
                    se = se_tiles.pop((hq, c))
                    nc.tensor.matmul(
                        dn[:, i, :], ones128[:], se[:], start=True, stop=True,
                    )
                return step

            def mk_norm(c2):
                def step():
                    py, dn = state["py"], state["dn"]
                    rb = rb_p.tile([128, 2, 256], f32, tag="rb")
                    nc.vector.reciprocal(out=rb[:], in_=dn[:])
                    nc.vector.tensor_mul(
                        yts[hq][:, ts(c2, 512)].rearrange(
                            "p (a b) -> p a b", a=2),
                        py[:], rb[:],
                    )
                return step

            out = []
            for c2 in range(2):
                out.append(mk_av(2 * c2))
                out.append(mk_av(2 * c2 + 1))
                out.append(mk_norm(c2))
            return out

        def interleave(primary, inserts):
            """Emit primary closures with inserts spread evenly between."""
            n, m = len(primary), len(inserts)
            ii = 0
            for k, p in enumerate(primary):
                p()
                want = (k + 1) * m // n
                while ii < want:
                    inserts[ii]()
                    ii += 1
            while ii < m:
                inserts[ii]()
                ii += 1

        # ================= PHASE A+B: qkv + interleaved attention =========
        with (
            tc.tile_pool(name="x", bufs=1) as x_p,
            tc.tile_pool(name="wv", bufs=4) as wv_p,
            tc.tile_pool(name="wqk", bufs=3) as wqk_p,
        ):
            # head order per group: [k_g, q_{4g}, .., q_{4g+3}]
            def head_list(g):
                return [NQ + g, 4 * g, 4 * g + 1, 4 * g + 2, 4 * g + 3]

            wqk_t: dict = {}

            def fetch_w(h, half):
                wqk_t[(h, half)] = wqk_p.tile(
                    [128, 8, 2, 128], f8, tag="wqk", name=f"wqk{h}_{half}")
                nc.sync.dma_start(
                    out=wqk_t[(h, half)][:], in_=wqk_d[h, :, ts(half, 8), :, :])

            # DMA priority order, staggered so the k0/v-tt startup chains
            # are fed as early as possible.
            fetch_w(head_list(0)[0], 0)
            xT_sb = x_p.tile([128, NCC, T], bf16)
            x8_sb = x_p.tile([128, NDR, 2, T], f8)
            xr = xT_d.rearrange("(cc p) t -> p cc t", p=128)

            def fetch_x8(ch):
                nc.sync.dma_start(
                    out=x8_sb[:, ts(ch, 2)], in_=x8_d[:, ts(ch, 2)]
                )

            def fetch_x(ch):
                nc.sync.dma_start(
                    out=xT_sb[:, ts(ch, 4), :], in_=xr[:, ts(ch, 4), :]
                )

            wv_t = {}

            def fetch_wv(ch):
                wv_t[ch] = wv_p.tile([128, 8, NKV * HD], bf16, tag="wv",
                                     name=f"wv{ch}")
                nc.sync.dma_start(out=wv_t[ch][:], in_=wv_d[:, ts(ch, 8), :])

            fetch_x8(0)
            fetch_x(0)
            fetch_wv(0)
            fetch_x8(1)
            fetch_x(1)
            fetch_w(head_list(0)[0], 1)
            fetch_x8(2)
            fetch_x(2)
            fetch_wv(1)
            fetch_x8(3)
            fetch_x(3)
            fetch_x8(4)
            fetch_x(4)
            fetch_wv(2)
            fetch_x8(5)
            fetch_x(5)
            fetch_x8(6)
            fetch_x(6)
            fetch_x8(7)
            fetch_x(7)
            fetch_wv(3)
            nc.sync.dma_start(out=csg[:], in_=csg_d[:])
            nc.sync.dma_start(out=ssg[:], in_=ssg_d[:])
            nc.sync.dma_start(out=mask_sb[:], in_=mask_d[:])

            # v psum chains: group-0 only -- borrow the score 3-ring and the
            # AV 2-ring (both idle during group 0) for up to 5 concurrent
            # chains.
            v_pool_cycle = [psSm, psSm, psSm, psYD, psYD]

            def v_steps(tt):
                """One v token-tile: 32-step accumulation + PSUM->SBUF copy."""
                pool = v_pool_cycle[tt % 5]
                psv = pool.tile(
                    [128, 512], f32, tag=("psS" if pool is psSm else "psY"),
                    name=f"psv{tt}")

                def mk_vcc(cc):
                    def step():
                        nc.tensor.matmul(
                            psv[:],
                            xT_sb[:, cc, ts(tt, 128)],
                            wv_t[cc // 8][:, cc % 8, :],
                            start=(cc == 0),
                            stop=(cc == NCC - 1),
                        )
                        if cc == NCC - 1:
                            nc.scalar.copy(out=v_sb[:, tt, :], in_=psv[:])
                    return step

                return [mk_vcc(cc) for cc in range(NCC)]

            def emit_slot(g, qk_steps, ins):
                if g == 0:
                    # per-chunk lockstep: all chains consume x chunk m
                    # before any chain touches chunk m+1 (DMA pacing).
                    # qk (DoubleRow halves) advances 4 steps per chunk;
                    # v chains advance 4 cc steps per chunk.
                    for ch in range(8):
                        for s in qk_steps[4 * ch : 4 * ch + 4]:
                            s()
                        for chain in ins:
                            for s in chain[4 * ch : 4 * ch + 4]:
                                s()
                else:
                    interleave(qk_steps, ins)

            # ---- q/k heads in groups; attention + v interleaved into slots
            def slot_inserts(g, j):
                ins = []
                if g == 0:
                    # v tiles: 5 chains ride with k0 (fills the DMA-paced
                    # startup window), the rest spread over the q slots.
                    vmap = {0: [0, 1, 2, 3, 4], 1: [5, 6], 2: [7], 3: [], 4: []}
                    return [v_steps(tt) for tt in vmap[j]]
                # attention of the previous group: head j hosted in slot j
                # (the j=4 slot stays single-loaded for g3's early heads)
                if g == 3 and j >= 3:
                    ins += avdn_steps(3, j - 3)
                if j <= 3:
                    ins += scores_steps(g - 1, j)
                    ins += avdn_steps(g - 1, j)
                if g == 3 and j >= 2:
                    ins += scores_steps(3, j - 2)
                return ins

            for g in range(4):
                heads = head_list(g)
                for j, h in enumerate(heads):
                    # prefetch: this head's second half + next head's first
                    if (h, 1) not in wqk_t:
                        fetch_w(h, 1)
                    nxt = heads[j + 1] if j + 1 < 5 else (
                        head_list(g + 1)[0] if g + 1 < 4 else None)
                    if nxt is not None:
                        fetch_w(nxt, 0)
                    psh = [psQK.tile([128, 512], f32, tag="psQK",
                                     name=f"ps{h}_{hf}") for hf in range(2)]
                    wt0 = wqk_t.pop((h, 0))
                    wt1 = wqk_t.pop((h, 1))
                    qt = qkT_p.tile([128, T], f8, tag="qkT", name=f"qkT{h}")

                    def rope_half(hf):
                        # RoPE on cols [512*hf, 512*hf+512):
                        # qt_h = ps_h*csg_h + swap_parts(ps_h)*ssg_h
                        ps = psh[hf]
                        cs = ts(hf, 512)
                        sw = sw_p.tile([128, 512], bf16, tag="sw")
                        nc.scalar.copy(out=sw[0:64, :], in_=ps[64:128, :])
                        nc.scalar.copy(out=sw[64:128, :], in_=ps[0:64, :])
                        m2 = m2_p.tile([128, 512], bf16, tag="m2")
                        nc.gpsimd.tensor_mul(m2[:], sw[:], ssg[:, cs])
                        nc.vector.tensor_mul(qt[:, cs], ps[:], csg[:, cs])
                        nc.vector.tensor_add(qt[:, cs], qt[:, cs], m2[:])

                    def mk_pass(jj, hf, wt, base):
                        def step():
                            nc.tensor.matmul(
                                psh[hf][:], wt[:, jj - base],
                                x8_sb[:, jj, :, ts(hf, 512)],
                                start=(jj == 0), stop=(jj == NDR - 1),
                                perf_mode=DRow,
                            )
                            if jj == NDR - 1:
                                rope_half(hf)
                        return step

                    if g == 0:
                        # chunk-locked order for DMA pacing
                        qk_steps = [mk_pass(jj, hf, wt0 if jj < 8 else wt1,
                                            0 if jj < 8 else 8)
                                    for jj in range(NDR) for hf in range(2)]
                    else:
                        # all of half 0 first: its RoPE overlaps half 1
                        qk_steps = [mk_pass(jj, hf, wt0 if jj < 8 else wt1,
                                            0 if jj < 8 else 8)
                                    for hf in range(2) for jj in range(NDR)]
                    emit_slot(g, qk_steps, slot_inserts(g, j))
                    qkT[h] = qt

        # ================= PHASE C: last-group attention + proj ===========
        with (
            tc.tile_pool(name="wpp", bufs=4) as wp_p,
            tc.tile_pool(name="ostage", bufs=3) as ostage_p,
        ):
            wp_t: dict = {}

            def fetch_wp(ccol):
                lo = wp_p.tile([128, 8, 512], bf16, tag="wp", name=f"wpl{ccol}")
                hi = wp_p.tile([128, 8, 512], bf16, tag="wp", name=f"wph{ccol}")
                nc.sync.dma_start(out=lo[:], in_=wp_d[ccol, :, 0:8, :])
                nc.sync.dma_start(out=hi[:], in_=wp_d[ccol, :, 8:16, :])
                wp_t[ccol] = (lo, hi)

            fetch_wp(0)

            def po_steps(ccol, tt, po, ycc_range):
                lo, hi = wp_t[ccol]

                def mk(ycc):
                    def step():
                        wtile = lo if ycc < 8 else hi
                        nc.tensor.matmul(
                            po[:],
                            yts[ycc][:, ts(tt, 128)],
                            wtile[:, ycc % 8, :],
                            start=(ycc == 0),
                            stop=(ycc == 15),
                        )
                        if ycc != 15:
                            return
                        ot = ostage_p.tile([128, 512], f32, tag="os")
                        nc.scalar.copy(out=ot[:], in_=po[:])
                        nc.sync.dma_start(
                            out=out_d[ts(tt, 128), ts(ccol, 512)],
                            in_=ot[:],
                        )
                    return step

                return [mk(ycc) for ycc in ycc_range]

            # remaining attention tail (the rest rode inside the B slots);
            # the first three proj accumulations' ycc 0-11 matmuls (which do
            # not depend on the last attention heads) fill the exp stalls.
            po0 = psQK.tile([128, 512], f32, tag="psQK", name="po0")
            po1 = psQK.tile([128, 512], f32, tag="psQK", name="po1")
            po2 = psQK.tile([128, 512], f32, tag="psQK", name="po2")
            tail = (avdn_steps(3, 2) + scores_steps(3, 3) + avdn_steps(3, 3))
            interleave(tail, po_steps(0, 0, po0, range(12))
                       + po_steps(0, 1, po1, range(12))
                       + po_steps(0, 2, po2, range(12)))

            first = {0: (po0, 12), 1: (po1, 12), 2: (po2, 12)}
            for ccol in range(8):
                if ccol + 1 < 8:
                    fetch_wp(ccol + 1)
                for tt in range(NTT):
                    po, ystart = None, 0
                    if ccol == 0 and tt in first:
                        po, ystart = first[tt]
                    else:
                        po = psQK.tile([128, 512], f32, tag="psQK", name="po")
                    for s in po_steps(ccol, tt, po, range(ystart, 16)):
                        s()
                wp_t.pop(ccol)

    nc.compile()
    return nc


def prep_inputs(x, Wqkv, Wproj, freqs_cos, freqs_sin):
    """Build the 8 per-core input maps (host-side shard + layout prep)."""
    x = np.asarray(x, np.float32)
    Wqkv = np.asarray(Wqkv, np.float32)
    Wproj = np.asarray(Wproj, np.float32)
    cos = np.asarray(freqs_cos, np.float32)
    sin = np.asarray(freqs_sin, np.float32)

    perm = np.concatenate([np.arange(0, HD, 2), np.arange(1, HD, 2)])
    csg = np.ascontiguousarray(
        (np.vstack([cos.T, cos.T]) / QK_FS2).astype(F8))   # [128, T]
    ssg = np.ascontiguousarray(
        (np.vstack([-sin.T, sin.T]) / QK_FS2).astype(F8))  # [128, T]
    # mask[p, i, f] = 1.0 if 128*i + p <= f else 0 (diagonal 256-chunk pair)
    mask = (
        (128 * np.arange(2)[None, :, None] + np.arange(128)[:, None, None])
        <= np.arange(256)[None, None, :]
    ).astype(BF16)
    mask = np.ascontiguousarray(mask)

    in_maps = []
    for c in range(8):
        b, hh = divmod(c, 2)
        qcols = (hh * NQ * HD + (np.arange(NQ) * HD)[:, None] + perm[None, :]).ravel()
        kcols = (
            H * HD + hh * NKV * HD + (np.arange(NKV) * HD)[:, None] + perm[None, :]
        ).ravel()
        vcols = (
            (H + KV) * HD
            + hh * NKV * HD
            + (np.arange(NKV) * HD)[:, None]
            + np.arange(HD)[None, :]
        ).ravel()
        Wqk = Wqkv[:, np.concatenate([qcols, kcols])]      # [4096, 2560]
        # fp8 DoubleRow layout [h, p, pass, i, col]:
        # contraction index c = 256*pass + 128*i + p
        wqk = np.ascontiguousarray(
            (QK_FS * Wqk).astype(F8)
            .reshape(NDR, 2, 128, QK_HEADS, 128).transpose(3, 2, 0, 1, 4))
        Wv = Wqkv[:, vcols]                                # [4096, 512]
        wv = np.ascontiguousarray(
            Wv.reshape(NCC, 128, NKV * HD).transpose(1, 0, 2).astype(BF16))
        Wp = Wproj[hh * NQ * HD : (hh + 1) * NQ * HD, :]   # [2048, 4096]
        wp = np.ascontiguousarray(
            Wp.reshape(16, 128, 8, 512).transpose(2, 1, 0, 3).astype(BF16))
        xT = np.ascontiguousarray(x[b].T.astype(BF16))     # [4096, 1024]
        x8 = np.ascontiguousarray(
            (QK_FS * x[b].T).astype(F8)
            .reshape(NDR, 2, 128, T).transpose(2, 0, 1, 3))  # [128, pass, i, T]
        in_maps.append(
            {"xT": xT, "x8": x8, "wqk": wqk, "wv": wv, "wp": wp,
             "csg": csg, "ssg": ssg, "maskd": mask}
        )
    return in_maps


def _get_nc():
    if "nc" not in _CACHE:
        _CACHE["nc"] = _build_nc()
    return _CACHE["nc"]


def kernel(x, Wqkv, Wproj, freqs_cos, freqs_sin, mask=None):
    from concourse.bass_utils import run_bass_kernel_spmd

    nc = _get_nc()
    in_maps = prep_inputs(x, Wqkv, Wproj, freqs_cos, freqs_sin)
    res = run_bass_kernel_spmd(nc, in_maps, core_ids=list(range(8)))
    outs = [res.results[c]["out"] for c in range(8)]
    y = np.stack([outs[2 * b] + outs[2 * b + 1] for b in range(B)], axis=0)
    return y.astype(np.float32)


# revision 16
# speedup vs baseline: 1.7745x; 1.3526x over previous
"""Causal self-attention (GQA + RoPE) Trainium2 Bass kernel, 8-way sharded.

Sharding: core c -> batch b = c // 2, head-half hh = c % 2.
Each core computes qkv projection, attention and output projection for its
batch and its 16 query heads / 4 kv heads; the output projection is a
row-shard of Wproj, so the two cores of a batch produce partial sums that
the host adds.

Design (all-SBUF):
  - q/k projection runs W-stationary in fp8-e4m3 DoubleRow (contraction 256
    per pass), emitting q^T/k^T [hd, T] directly. The fp8 quantization of
    x/Wqk/cos/sin/qkT only perturbs attention *scores* (softmax weights move
    by ~1e-4 relative for this data regime), so it is error-neutral; v and
    the projections stay bf16. Scale bookkeeping: x8 = fp8(64 x),
    w8 = fp8(64 W), csg/ssg carry 1/8, so qkT = 512 q_rope (fits e4m3
    range) and exp() divides by 512^2.
  - RoPE is applied per 512-col half on the PSUM->SBUF path: ACT (half
    swap) + Pool (sw*ssg) + DVE (ps*csg into qt, then += m2).
  - each q/k head accumulates its two 512-col halves in separate 1-bank
    PSUM tiles from a 3-ring, so RoPE of half 0 overlaps accumulation of
    half 1 and heads pipeline without bank stalls.
  - v runs x-stationary (bf16) into natural [T, 4*HD] layout; in group 0
    its 8 psum chains run up to 5-wide (3-ring + 2-ring banks) in per-x-
    chunk lockstep with k0, absorbing the DMA-paced startup window.
  - attention uses transposed scores (scoresT = kT_slice^T @ qT). Softmax:
    exp on ACT (PSUM->SBUF, bf16), si-pair exp tiles tree-summed on DVE,
    then ONE ones-matrix matmul per 256-col chunk yields the denominator
    broadcast across partitions. Score psums draw from a 3-ring; AV output
    and the denominator alternate in a separate 2-ring, so score matmuls
    never wait on the AV/normalize pipeline's banks.
  - attention for kv-group g is emitted interleaved into the qkv slots of
    group g+1; group 3's tail overlaps the first three projection
    accumulations (ycc 0-11).
  - projection reads SBUF-resident bf16 y^T tiles; PSUM accumulation fp32.
"""

import os

os.environ.setdefault("JAX_PLATFORMS", "axon")

import numpy as np
import ml_dtypes

BF16 = ml_dtypes.bfloat16
F8 = ml_dtypes.float8_e4m3

B, T, C = 4, 1024, 4096
H, KV, HD = 32, 8, 128
REP = H // KV  # 4

NQ = 16      # q heads per core
NKV = 4      # kv heads per core
QK_HEADS = NQ + NKV   # 20 projected+rope'd heads per core
NCC = C // 128        # 32 contraction tiles
NDR = C // 256        # 16 DoubleRow passes (256-contraction each)
NTT = T // 128        # 8 token tiles
SCALE = float(1.0 / np.sqrt(np.float32(HD)).astype(np.float32))
# fp8 scaling for the q/k projection: x and Wqk are multiplied by QK_FS
# before e4m3 quantization; qkT carries QK_FS**2/QK_FS2; exp() divides the
# product of two such factors back out.
QK_FS = 64.0
QK_FS2 = 8.0
EXP_SCALE = float(SCALE * QK_FS2**2 / QK_FS**4)

_CACHE: dict = {}


def _build_nc():
    import concourse.mybir as mybir
    import concourse.tile as tile
    from concourse import bacc
    from concourse.bass import ts

    f32 = mybir.dt.float32
    bf16 = mybir.dt.bfloat16
    f8 = mybir.dt.float8e4
    DRow = mybir.MatmulPerfMode.DoubleRow
    Exp = mybir.ActivationFunctionType.Exp

    nc = bacc.Bacc(None, target_bir_lowering=False, debug=False)

    xT_d = nc.dram_tensor("xT", [C, T], bf16, kind="ExternalInput")
    # fp8 x for the q/k projection, pre-paired for DoubleRow:
    # x8[p, j, i, t] = fp8(QK_FS * x[t, 256*j + 128*i + p])
    x8_d = nc.dram_tensor("x8", [128, NDR, 2, T], f8, kind="ExternalInput")
    # [h, p(c within pair), pass, i, col]
    wqk_d = nc.dram_tensor("wqk", [QK_HEADS, 128, NDR, 2, 128], f8,
                           kind="ExternalInput")
    # [p, cc, vcol]
    wv_d = nc.dram_tensor("wv", [128, NCC, NKV * HD], bf16, kind="ExternalInput")
    # [ccol, p(y within tile), ycc, f]
    wp_d = nc.dram_tensor("wp", [8, 128, 16, 512], bf16, kind="ExternalInput")
    csg_d = nc.dram_tensor("csg", [128, T], f8, kind="ExternalInput")
    ssg_d = nc.dram_tensor("ssg", [128, T], f8, kind="ExternalInput")
    mask_d = nc.dram_tensor("maskd", [128, 2, 256], bf16, kind="ExternalInput")
    out_d = nc.dram_tensor("out", [T, C], f32, kind="ExternalOutput")

    with (
        tile.TileContext(nc) as tc,
        tc.tile_pool(name="const", bufs=1) as const_p,
        tc.tile_pool(name="qkT", bufs=10) as qkT_p,
        tc.tile_pool(name="yts", bufs=NQ) as yts_p,
        tc.tile_pool(name="vsb", bufs=1) as vsb_p,
        tc.tile_pool(name="sw", bufs=2) as sw_p,
        tc.tile_pool(name="m2", bufs=2) as m2_p,
        tc.tile_pool(name="et", bufs=10) as et_p,
        tc.tile_pool(name="se", bufs=5) as se_p,
        tc.tile_pool(name="t2", bufs=2) as t2_p,
        tc.tile_pool(name="rb", bufs=1) as rb_p,
        # PSUM bank map (8 banks total):
        #   psQK: 3 x 1 bank -- q/k half-accumulators; proj po chains
        #   psSm: 3 x 1 bank -- score psums; v chains (g0); po0-2 tail
        #   psYD: 2 x 1 bank -- AV output / softmax denominator; v chains
        tc.tile_pool(name="psQK", bufs=3, space="PSUM") as psQK,
        tc.tile_pool(name="psSm", bufs=3, space="PSUM") as psSm,
        tc.tile_pool(name="psYD", bufs=2, space="PSUM") as psYD,
    ):
        ones128 = const_p.tile([128, 128], bf16)
        nc.vector.memset(ones128[:], 1.0)
        csg = const_p.tile([128, T], f8)
        ssg = const_p.tile([128, T], f8)
        mask_sb = const_p.tile([128, 2, 256], bf16)

        v_sb = vsb_p.tile([128, NTT, NKV * HD], bf16)
        yts = [yts_p.tile([128, T], bf16, tag="yt", name=f"yt{i}")
               for i in range(NQ)]
        qkT: dict = {}

        # ---- attention emission helpers (interleaved into qkv slots) ----
        et_tiles: dict = {}
        se_tiles: dict = {}

        def scores_steps(g, r):
            """Emission-step closures: scoresT + exp + mask + pair-tree sums
            for q head (g, r). One step per si pair (10 total)."""
            hq = 4 * g + r
            ew = nc.vector

            def mk(c, pair):
                def step():
                    qt = qkT[hq]
                    kt = qkT[NQ + g]
                    np_ = c + 1
                    pss = psSm.tile([128, 2, 256], f32, tag="psS")
                    et = et_p.tile([128, 2, 256], bf16, tag="et")
                    # both si blocks full-width; the diagonal pair's masked
                    # region is computed then zeroed by the mask multiply
                    for i in range(2):
                        si = 2 * pair + i
                        nc.tensor.matmul(
                            pss[:, i, :],
                            kt[:, ts(si, 128)],
                            qt[:, ts(c, 256)],
                            start=True,
                            stop=True,
                        )
                    nc.scalar.activation(
                        out=et[:], in_=pss[:], func=Exp, scale=EXP_SCALE,
                    )
                    pairs = et_tiles.setdefault((hq, c), [])
                    pairs.append(et)
                    if pair == c:  # diagonal pair: mask, then finish the sum
                        ew.tensor_mul(et[:], et[:], mask_sb[:])
                        se = se_p.tile([128, 256], bf16, tag="se")
                        if np_ == 1:
                            ew.tensor_add(se[:], et[:, 0, :], et[:, 1, :])
                        else:
                            t2 = t2_p.tile([128, 2, 256], bf16, tag="t2")
                            ew.tensor_add(t2[:], pairs[0][:], pairs[1][:])
                            for k in range(2, np_):
                                ew.tensor_add(t2[:], t2[:], pairs[k][:])
                            ew.tensor_add(se[:], t2[:, 0, :], t2[:, 1, :])
                        se_tiles[(hq, c)] = se
                return step

            return [mk(c, pair) for c in range(4) for pair in range(c + 1)]

        def avdn_steps(g, r):
            """Emission-step closures: AV + broadcast denominator + normalize
            for q head (g, r). One step per chunk plus one per chunk-pair."""
            hq = 4 * g + r
            state: dict = {}

            def mk_av(c):
                def step():
                    c2, i = divmod(c, 2)
                    if i == 0:
                        state["py"] = psYD.tile([128, 2, 256], f32, tag="psY",
                                                name="py")
                        state["dn"] = psYD.tile([128, 2, 256], f32, tag="psY",
                                                name="dn")
                    py, dn = state["py"], state["dn"]
                    ns = 2 * (c + 1)
                    pairs = et_tiles.pop((hq, c))
                    for si in range(ns):
                        if si == ns - 1:
                            # odd-diagonal block: lower 128 tq cols are zero
                            nc.tensor.matmul(
                                py[:, i, 128:256],
                                v_sb[:, si, ts(g, 128)],
                                pairs[si // 2][:, si % 2, 128:256],
                                start=False,
                                stop=True,
                            )
                            continue
                        nc.tensor.matmul(
                            py[:, i, :],
                            v_sb[:, si, ts(g, 128)],
                            pairs[si // 2][:, si % 2, :],
                            start=(si == 0),
                            stop=False,
                        )
                    se = se_tiles.pop((hq, c))
                    nc.tensor.matmul(
                        dn[:, i, :], ones128[:], se[:], start=True, stop=True,
                    )
                return step

            def mk_norm(c2):
                def step():
                    py, dn = state["py"], state["dn"]
                    rb = rb_p.tile([128, 2, 256], f32, tag="rb")
                    nc.vector.reciprocal(out=rb[:], in_=dn[:])
                    nc.vector.tensor_mul(
                        yts[hq][:, ts(c2, 512)].rearrange(
                            "p (a b) -> p a b", a=2),
                        py[:], rb[:],
                    )
                return step

            out = []
            for c2 in range(2):
                out.append(mk_av(2 * c2))
                out.append(mk_av(2 * c2 + 1))
                out.append(mk_norm(c2))
            return out

        def interleave(primary, inserts):
            """Emit primary closures with inserts spread evenly between."""
            n, m = len(primary), len(inserts)
            ii = 0
            for k, p in enumerate(primary):
                p()
                want = (k + 1) * m // n
                while ii < want:
                    inserts[ii]()
                    ii += 1
            while ii < m:
                inserts[ii]()
                ii += 1

        # ================= PHASE A+B: qkv + interleaved attention =========
        with (
            tc.tile_pool(name="x", bufs=1) as x_p,
            tc.tile_pool(name="wv", bufs=4) as wv_p,
            tc.tile_pool(name="wqk", bufs=3) as wqk_p,
        ):
            # head order per group: [k_g, q_{4g}, .., q_{4g+3}]
            def head_list(g):
                return [NQ + g, 4 * g, 4 * g + 1, 4 * g + 2, 4 * g + 3]

            wqk_t: dict = {}

            def fetch_w(h, half):
                wqk_t[(h, half)] = wqk_p.tile(
                    [128, 8, 2, 128], f8, tag="wqk", name=f"wqk{h}_{half}")
                nc.sync.dma_start(
                    out=wqk_t[(h, half)][:], in_=wqk_d[h, :, ts(half, 8), :, :])

            # DMA priority order, staggered so the k0/v-tt startup chains
            # are fed as early as possible.
            fetch_w(head_list(0)[0], 0)
            xT_sb = x_p.tile([128, NCC, T], bf16)
            x8_sb = x_p.tile([128, NDR, 2, T], f8)
            xr = xT_d.rearrange("(cc p) t -> p cc t", p=128)

            def fetch_x8(ch):
                nc.sync.dma_start(
                    out=x8_sb[:, ts(ch, 2)], in_=x8_d[:, ts(ch, 2)]
                )

            def fetch_x(ch):
                nc.sync.dma_start(
                    out=xT_sb[:, ts(ch, 4), :], in_=xr[:, ts(ch, 4), :]
                )

            wv_t = {}

            def fetch_wv(ch):
                wv_t[ch] = wv_p.tile([128, 8, NKV * HD], bf16, tag="wv",
                                     name=f"wv{ch}")
                nc.sync.dma_start(out=wv_t[ch][:], in_=wv_d[:, ts(ch, 8), :])

            fetch_x8(0)
            fetch_wv(0)
            fetch_x(0)
            fetch_x8(1)
            fetch_x(1)
            fetch_w(head_list(0)[0], 1)
            fetch_x8(2)
            fetch_wv(1)
            fetch_x(2)
            fetch_x8(3)
            fetch_x(3)
            fetch_x8(4)
            fetch_x(4)
            fetch_wv(2)
            fetch_x8(5)
            fetch_x(5)
            fetch_x8(6)
            fetch_x(6)
            fetch_x8(7)
            fetch_x(7)
            fetch_wv(3)
            nc.sync.dma_start(out=csg[:], in_=csg_d[:])
            nc.sync.dma_start(out=ssg[:], in_=ssg_d[:])
            nc.sync.dma_start(out=mask_sb[:], in_=mask_d[:])

            # v psum chains: group-0 only -- borrow the score 3-ring and the
            # AV 2-ring (both idle during group 0) for up to 5 concurrent
            # chains.
            v_pool_cycle = [psSm, psSm, psSm, psYD, psYD]

            def v_steps(tt):
                """One v token-tile: 32-step accumulation + PSUM->SBUF copy."""
                pool = v_pool_cycle[tt % 5]
                psv = pool.tile(
                    [128, 512], f32, tag=("psS" if pool is psSm else "psY"),
                    name=f"psv{tt}")

                def mk_vcc(cc):
                    def step():
                        nc.tensor.matmul(
                            psv[:],
                            xT_sb[:, cc, ts(tt, 128)],
                            wv_t[cc // 8][:, cc % 8, :],
                            start=(cc == 0),
                            stop=(cc == NCC - 1),
                        )
                        if cc == NCC - 1:
                            nc.scalar.copy(out=v_sb[:, tt, :], in_=psv[:])
                    return step

                return [mk_vcc(cc) for cc in range(NCC)]

            def emit_slot(g, qk_steps, ins):
                if g == 0:
                    # per-chunk lockstep: all chains consume x chunk m
                    # before any chain touches chunk m+1 (DMA pacing).
                    # qk (DoubleRow halves) advances 4 steps per chunk;
                    # v chains advance 4 cc steps per chunk.
                    for ch in range(8):
                        for s in qk_steps[4 * ch : 4 * ch + 4]:
                            s()
                        for chain in ins:
                            for s in chain[4 * ch : 4 * ch + 4]:
                                s()
                else:
                    interleave(qk_steps, ins)

            # ---- q/k heads in groups; attention + v interleaved into slots
            def slot_inserts(g, j):
                ins = []
                if g == 0:
                    # v tiles: 5 chains ride with k0 (fills the DMA-paced
                    # startup window), the rest spread over the q slots.
                    vmap = {0: [0, 1, 2, 3, 4], 1: [5, 6], 2: [7], 3: [], 4: []}
                    return [v_steps(tt) for tt in vmap[j]]
                # attention of the previous group: head j hosted in slot j
                # (the j=4 slot stays single-loaded for g3's early heads)
                if g == 3 and j >= 3:
                    ins += avdn_steps(3, j - 3)
                if j <= 3:
                    ins += scores_steps(g - 1, j)
                    ins += avdn_steps(g - 1, j)
                if g == 3 and j >= 2:
                    ins += scores_steps(3, j - 2)
                return ins

            for g in range(4):
                heads = head_list(g)
                for j, h in enumerate(heads):
                    # prefetch: this head's second half + next head's first
                    if (h, 1) not in wqk_t:
                        fetch_w(h, 1)
                    nxt = heads[j + 1] if j + 1 < 5 else (
                        head_list(g + 1)[0] if g + 1 < 4 else None)
                    if nxt is not None:
                        fetch_w(nxt, 0)
                    psh = [psQK.tile([128, 512], f32, tag="psQK",
                                     name=f"ps{h}_{hf}") for hf in range(2)]
                    wt0 = wqk_t.pop((h, 0))
                    wt1 = wqk_t.pop((h, 1))
                    qt = qkT_p.tile([128, T], f8, tag="qkT", name=f"qkT{h}")

                    def rope_half(hf):
                        # RoPE on cols [512*hf, 512*hf+512):
                        # qt_h = ps_h*csg_h + swap_parts(ps_h)*ssg_h
                        ps = psh[hf]
                        cs = ts(hf, 512)
                        sw = sw_p.tile([128, 512], bf16, tag="sw")
                        nc.scalar.copy(out=sw[0:64, :], in_=ps[64:128, :])
                        nc.scalar.copy(out=sw[64:128, :], in_=ps[0:64, :])
                        m2 = m2_p.tile([128, 512], bf16, tag="m2")
                        nc.gpsimd.tensor_mul(m2[:], sw[:], ssg[:, cs])
                        nc.vector.tensor_mul(qt[:, cs], ps[:], csg[:, cs])
                        nc.vector.tensor_add(qt[:, cs], qt[:, cs], m2[:])

                    def mk_pass(jj, hf, wt, base):
                        def step():
                            nc.tensor.matmul(
                                psh[hf][:], wt[:, jj - base],
                                x8_sb[:, jj, :, ts(hf, 512)],
                                start=(jj == 0), stop=(jj == NDR - 1),
                                perf_mode=DRow,
                            )
                            if jj == NDR - 1:
                                rope_half(hf)
                        return step

                    if g == 0:
                        # chunk-locked order for DMA pacing
                        qk_steps = [mk_pass(jj, hf, wt0 if jj < 8 else wt1,
                                            0 if jj < 8 else 8)
                                    for jj in range(NDR) for hf in range(2)]
                    else:
                        # all of half 0 first: its RoPE overlaps half 1
                        qk_steps = [mk_pass(jj, hf, wt0 if jj < 8 else wt1,
                                            0 if jj < 8 else 8)
                                    for hf in range(2) for jj in range(NDR)]
                    emit_slot(g, qk_steps, slot_inserts(g, j))
                    qkT[h] = qt

        # ================= PHASE C: last-group attention + proj ===========
        with (
            tc.tile_pool(name="wpp", bufs=4) as wp_p,
            tc.tile_pool(name="ostage", bufs=3) as ostage_p,
        ):
            wp_t: dict = {}

            def fetch_wp(ccol):
                lo = wp_p.tile([128, 8, 512], bf16, tag="wp", name=f"wpl{ccol}")
                hi = wp_p.tile([128, 8, 512], bf16, tag="wp", name=f"wph{ccol}")
                nc.sync.dma_start(out=lo[:], in_=wp_d[ccol, :, 0:8, :])
                nc.sync.dma_start(out=hi[:], in_=wp_d[ccol, :, 8:16, :])
                wp_t[ccol] = (lo, hi)

            fetch_wp(0)

            def po_steps(ccol, tt, po, ycc_range):
                lo, hi = wp_t[ccol]

                def mk(ycc):
                    def step():
                        wtile = lo if ycc < 8 else hi
                        nc.tensor.matmul(
                            po[:],
                            yts[ycc][:, ts(tt, 128)],
                            wtile[:, ycc % 8, :],
                            start=(ycc == 0),
                            stop=(ycc == 15),
                        )
                        if ycc != 15:
                            return
                        ot = ostage_p.tile([128, 512], f32, tag="os")
                        nc.scalar.copy(out=ot[:], in_=po[:])
                        nc.sync.dma_start(
                            out=out_d[ts(tt, 128), ts(ccol, 512)],
                            in_=ot[:],
                        )
                    return step

                return [mk(ycc) for ycc in ycc_range]

            # remaining attention tail (the rest rode inside the B slots);
            # the first three proj accumulations' ycc 0-11 matmuls (which do
            # not depend on the last attention heads) fill the exp stalls.
            po0 = psQK.tile([128, 512], f32, tag="psQK", name="po0")
            po1 = psQK.tile([128, 512], f32, tag="psQK", name="po1")
            po2 = psQK.tile([128, 512], f32, tag="psQK", name="po2")
            tail = (avdn_steps(3, 2) + scores_steps(3, 3) + avdn_steps(3, 3))
            interleave(tail, po_steps(0, 0, po0, range(12))
                       + po_steps(0, 1, po1, range(12))
                       + po_steps(0, 2, po2, range(12)))

            first = {0: (po0, 12), 1: (po1, 12), 2: (po2, 12)}
            for ccol in range(8):
                if ccol + 1 < 8:
                    fetch_wp(ccol + 1)
                for tt in range(NTT):
                    po, ystart = None, 0
                    if ccol == 0 and tt in first:
                        po, ystart = first[tt]
                    else:
                        po = psQK.tile([128, 512], f32, tag="psQK", name="po")
                    for s in po_steps(ccol, tt, po, range(ystart, 16)):
                        s()
                wp_t.pop(ccol)

    nc.compile()
    return nc


def prep_inputs(x, Wqkv, Wproj, freqs_cos, freqs_sin):
    """Build the 8 per-core input maps (host-side shard + layout prep)."""
    x = np.asarray(x, np.float32)
    Wqkv = np.asarray(Wqkv, np.float32)
    Wproj = np.asarray(Wproj, np.float32)
    cos = np.asarray(freqs_cos, np.float32)
    sin = np.asarray(freqs_sin, np.float32)

    perm = np.concatenate([np.arange(0, HD, 2), np.arange(1, HD, 2)])
    csg = np.ascontiguousarray(
        (np.vstack([cos.T, cos.T]) / QK_FS2).astype(F8))   # [128, T]
    ssg = np.ascontiguousarray(
        (np.vstack([-sin.T, sin.T]) / QK_FS2).astype(F8))  # [128, T]
    # mask[p, i, f] = 1.0 if 128*i + p <= f else 0 (diagonal 256-chunk pair)
    mask = (
        (128 * np.arange(2)[None, :, None] + np.arange(128)[:, None, None])
        <= np.arange(256)[None, None, :]
    ).astype(BF16)
    mask = np.ascontiguousarray(mask)

    in_maps = []
    for c in range(8):
        b, hh = divmod(c, 2)
        qcols = (hh * NQ * HD + (np.arange(NQ) * HD)[:, None] + perm[None, :]).ravel()
        kcols = (
            H * HD + hh * NKV * HD + (np.arange(NKV) * HD)[:, None] + perm[None, :]
        ).ravel()
        vcols = (
            (H + KV) * HD
            + hh * NKV * HD
            + (np.arange(NKV) * HD)[:, None]
            + np.arange(HD)[None, :]
        ).ravel()
        Wqk = Wqkv[:, np.concatenate([qcols, kcols])]      # [4096, 2560]
        # fp8 DoubleRow layout [h, p, pass, i, col]:
        # contraction index c = 256*pass + 128*i + p
        wqk = np.ascontiguousarray(
            (QK_FS * Wqk).astype(F8)
            .reshape(NDR, 2, 128, QK_HEADS, 128).transpose(3, 2, 0, 1, 4))
        Wv = Wqkv[:, vcols]                                # [4096, 512]
        wv = np.ascontiguousarray(
            Wv.reshape(NCC, 128, NKV * HD).transpose(1, 0, 2).astype(BF16))
        Wp = Wproj[hh * NQ * HD : (hh + 1) * NQ * HD, :]   # [2048, 4096]
        wp = np.ascontiguousarray(
            Wp.reshape(16, 128, 8, 512).transpose(2, 1, 0, 3).astype(BF16))
        xT = np.ascontiguousarray(x[b].T.astype(BF16))     # [4096, 1024]
        x8 = np.ascontiguousarray(
            (QK_FS * x[b].T).astype(F8)
            .reshape(NDR, 2, 128, T).transpose(2, 0, 1, 3))  # [128, pass, i, T]
        in_maps.append(
            {"xT": xT, "x8": x8, "wqk": wqk, "wv": wv, "wp": wp,
             "csg": csg, "ssg": ssg, "maskd": mask}
        )
    return in_maps


def _get_nc():
    if "nc" not in _CACHE:
        _CACHE["nc"] = _build_nc()
    return _CACHE["nc"]


def kernel(x, Wqkv, Wproj, freqs_cos, freqs_sin, mask=None):
    from concourse.bass_utils import run_bass_kernel_spmd

    nc = _get_nc()
    in_maps = prep_inputs(x, Wqkv, Wproj, freqs_cos, freqs_sin)
    res = run_bass_kernel_spmd(nc, in_maps, core_ids=list(range(8)))
    outs = [res.results[c]["out"] for c in range(8)]
    y = np.stack([outs[2 * b] + outs[2 * b + 1] for b in range(B)], axis=0)
    return y.astype(np.float32)
